# revision 6
# baseline (speedup 1.0000x reference)
"""Trainium2 Bass kernel for nn_GaussianSelfAttention (B=64, S=197, D=768).

Math: the reference's softmax is over a singleton axis, so attn == 1.0 exactly
and out = concat([ones(B,1,D), sample_v], axis=1) where
sample_v = (G @ x) @ Wv + wsum*bv,  G = per-image (196,197) bilinear one-hot
matrix built from Gaussian-sampled keys. q/k projections are dead code.

Device strategy (8 cores, data-parallel over batch, 8 images/core):
  - key/weight/index math on DVE in fp32 (exact floor via int-roundtrip)
  - one-hot rows built with fused tensor_scalar(is_equal, mult)
  - 4 point-set one-hots combined via accumulating PE transposes -> GT
  - sxT = x.T-gather via matmul(lhsT=x, rhs=GT) in f32r (N padded to 256)
  - sv = sxT.T @ Wv in f32r, evacuated and DMA'd to the output rows
"""

import numpy as np

import concourse.bass as bass
import concourse.mybir as mybir
import concourse.tile as tile
from concourse import bacc, bass_utils
from concourse.masks import make_identity

B, S, D, P = 64, 197, 768, 196
N_CORES = 8
BPC = B // N_CORES            # images per core
ROWS = BPC * S                # 1576 input/output rows per core
Q = BPC * P                   # 1568 sampled rows per core
GRID = 14.0
NF = 198                      # padded one-hot free size (even, >= S)
NPAD = 256                    # padded gather-matmul N (f32r full rate)

F32 = mybir.dt.float32
F32R = mybir.dt.float32r
I32 = mybir.dt.int32
OP = mybir.AluOpType

PCH = [(0, 128), (128, 68)]   # partition chunks of P=196
SCH = [(0, 128), (128, 69)]   # chunks of S=197

_NC = None


def _emit(nc):
    x_d = nc.dram_tensor("x0", (ROWS, D), F32R, kind="ExternalInput")
    wv_d = nc.dram_tensor("wv0", (D, D), F32R, kind="ExternalInput")
    pr_d = nc.dram_tensor("pr0", (P, 6 * BPC), F32, kind="ExternalInput")
    o_d = nc.dram_tensor("o0", (ROWS, D), F32, kind="ExternalOutput")

    with tile.TileContext(nc) as tc:
        with (
            tc.tile_pool(name="const", bufs=1) as cpool,
            tc.tile_pool(name="xb", bufs=1) as xpool,
            tc.tile_pool(name="wvp", bufs=1) as wpool,
            tc.tile_pool(name="sxp", bufs=1) as spool,
            tc.tile_pool(name="km", bufs=1) as kpool,
            tc.tile_pool(name="gp", bufs=2) as gpool,
            tc.tile_pool(name="gtp", bufs=2) as gtpool,
            tc.tile_pool(name="ost", bufs=3) as opool,
            tc.tile_pool(name="psT", bufs=2, space="PSUM") as psT,
            tc.tile_pool(name="psA", bufs=3, space="PSUM") as psA,
            tc.tile_pool(name="psB", bufs=3, space="PSUM") as psB,
        ):
            # ---- constants ----
            ident = cpool.tile([128, 128], F32, name="ident", tag="ident")
            make_identity(nc, ident[:])
            iotaf = cpool.tile([128, NF], F32, name="iota", tag="iota")
            nc.gpsimd.iota(iotaf[:], pattern=[[1, NF]], base=0,
                           channel_multiplier=0,
                           allow_small_or_imprecise_dtypes=True)
            ones = cpool.tile([BPC, D], F32, name="ones", tag="ones")
            nc.vector.memset(ones[:], 1.0)
            zpad = cpool.tile([128, NPAD - P], F32, name="zpad", tag="zpad")
            nc.vector.memset(zpad[:], 0.0)

            # ---- weights / activations ----
            wv_sb = []
            for kc in range(6):
                t = wpool.tile([128, D], F32R, name=f"wv{kc}", tag=f"wv{kc}")
                nc.sync.dma_start(out=t[:], in_=wv_d[kc * 128:(kc + 1) * 128, :])
                wv_sb.append(t)
            xt = []
            for b in range(BPC):
                r0 = b * S
                t0 = xpool.tile([128, D], F32R, name=f"x{b}_0", tag=f"x{b}_0")
                nc.sync.dma_start(out=t0[:], in_=x_d[r0:r0 + 128, :])
                t1 = xpool.tile([69, D], F32R, name=f"x{b}_1", tag=f"x{b}_1")
                nc.sync.dma_start(out=t1[:], in_=x_d[r0 + 128:r0 + S, :])
                xt.append((t0, t1))
            sxT = [spool.tile([128, Q], F32R, name=f"sxT{kc}", tag=f"sxT{kc}") for kc in range(6)]

            # ---- key / weight / index math (per partition chunk of P) ----
            # pr layout: [p, j*BPC + b], j: 0 nx 1 ny 2 ax 3 ay 4 sx 5 sy
            w4 = []
            i4 = []
            for c, (p0, pn) in enumerate(PCH):
                pr = kpool.tile([pn, 6 * BPC], F32, name=f"pr{c}", tag=f"pr{c}")
                nc.sync.dma_start(out=pr[:], in_=pr_d[p0:p0 + pn, :])

                def col(j):
                    return pr[:, j * BPC:(j + 1) * BPC]

                def tl(tag):
                    return kpool.tile([pn, BPC], F32, name=f"{tag}{c}", tag=f"{tag}{c}")

                # keys: k = (noise - a) * (1/s)
                keys = []
                for (jn, ja, js) in ((0, 2, 4), (1, 3, 5)):
                    k = tl(f"k{jn}")
                    nc.vector.tensor_tensor(out=k[:], in0=col(jn), in1=col(ja),
                                            op=OP.subtract)
                    rs = tl(f"rs{jn}")
                    nc.vector.reciprocal(rs[:], col(js))
                    nc.vector.tensor_tensor(out=k[:], in0=k[:], in1=rs[:],
                                            op=OP.mult)
                    keys.append(k)
                kx, ky = keys

                # floor/ceil via exact int roundtrip: tf = f32(i32(k)) (RNE)
                # flo = tf - (tf > k); cei = flo + (k > flo)
                cells = {}
                for nm, k in (("x", kx), ("y", ky)):
                    ti = kpool.tile([pn, BPC], I32, name=f"ti{nm}{c}", tag=f"ti{nm}{c}")
                    nc.vector.tensor_copy(out=ti[:], in_=k[:])
                    tf = tl(f"tf{nm}")
                    nc.vector.tensor_copy(out=tf[:], in_=ti[:])
                    corr = tl(f"co{nm}")
                    nc.vector.tensor_tensor(out=corr[:], in0=tf[:], in1=k[:],
                                            op=OP.is_gt)
                    flo = tl(f"fl{nm}")
                    nc.vector.tensor_tensor(out=flo[:], in0=tf[:], in1=corr[:],
                                            op=OP.subtract)
                    up = tl(f"up{nm}")
                    nc.vector.tensor_tensor(out=up[:], in0=k[:], in1=flo[:],
                                            op=OP.is_gt)
                    cei = tl(f"ce{nm}")
                    nc.vector.tensor_tensor(out=cei[:], in0=flo[:], in1=up[:],
                                            op=OP.add)
                    # bilinear 1D weights: wc = 1-(cei-k) = 1-|cei-k|, wf = 1-(k-flo)
                    dc = tl(f"dc{nm}")
                    nc.vector.tensor_tensor(out=dc[:], in0=cei[:], in1=k[:],
                                            op=OP.subtract)
                    wc = tl(f"wc{nm}")
                    nc.vector.tensor_scalar(out=wc[:], in0=dc[:], scalar1=-1.0,
                                            scalar2=1.0, op0=OP.mult, op1=OP.add)
                    df = tl(f"df{nm}")
                    nc.vector.tensor_tensor(out=df[:], in0=k[:], in1=flo[:],
                                            op=OP.subtract)
                    wf = tl(f"wf{nm}")
                    nc.vector.tensor_scalar(out=wf[:], in0=df[:], scalar1=-1.0,
                                            scalar2=1.0, op0=OP.mult, op1=OP.add)
                    cells[nm] = (cei, flo, wc, wf)

                x1, x2, wx1, wx2 = cells["x"]
                y1, y2, wy1, wy2 = cells["y"]

                # combos in reference order: 11=(x1,y1) 21=(x2,y1) 12=(x1,y2) 22=(x2,y2)
                w4c = kpool.tile([pn, 4 * BPC], F32, name=f"w4{c}", tag=f"w4{c}")
                i4c = kpool.tile([pn, 4 * BPC], F32, name=f"i4{c}", tag=f"i4{c}")
                fy = {}
                for nm, yy in (("y1", y1), ("y2", y2)):
                    f = tl(f"fy{nm}")
                    nc.vector.tensor_scalar(out=f[:], in0=yy[:], scalar1=GRID,
                                            scalar2=None, op0=OP.mult)
                    fy[nm] = f
                combos = [(x1, wx1, "y1", wy1), (x2, wx2, "y1", wy1),
                          (x1, wx1, "y2", wy2), (x2, wx2, "y2", wy2)]
                for ci, (xx, wxx, ynm, wyy) in enumerate(combos):
                    sl = slice(ci * BPC, (ci + 1) * BPC)
                    nc.vector.tensor_tensor(out=w4c[:, sl], in0=wxx[:],
                                            in1=wyy[:], op=OP.mult)
                    f = tl(f"f{ci}")
                    nc.vector.tensor_tensor(out=f[:], in0=fy[ynm][:], in1=xx[:],
                                            op=OP.add)
                    # wrap negatives: idx = f + 197*(f<0)  (f integral, |f|<197)
                    wr = tl(f"wr{ci}")
                    nc.vector.tensor_scalar(out=wr[:], in0=f[:], scalar1=0.0,
                                            scalar2=float(S), op0=OP.is_lt,
                                            op1=OP.mult)
                    nc.vector.tensor_tensor(out=i4c[:, sl], in0=f[:], in1=wr[:],
                                            op=OP.add)
                w4.append(w4c)
                i4.append(i4c)

            # ---- per image: one-hots -> accumulated transposes -> GT ----
            gts = []
            for b in range(BPC):
                gcs = []  # per chunk: 4 weighted one-hot tiles (pn, NF)
                for c, (p0, pn) in enumerate(PCH):
                    g4 = []
                    for ci in range(4):
                        g = gpool.tile([pn, NF], F32, name=f"g{c}_{ci}", tag=f"g{c}_{ci}")
                        nc.vector.tensor_scalar(
                            out=g[:], in0=iotaf[:pn, :],
                            scalar1=i4[c][:, ci * BPC + b:ci * BPC + b + 1],
                            scalar2=w4[c][:, ci * BPC + b:ci * BPC + b + 1],
                            op0=OP.is_equal, op1=OP.mult)
                        g4.append(g)
                    gcs.append(g4)

                gt0 = gtpool.tile([128, NPAD], F32R, name="gt0", tag="gt0")
                gt1 = gtpool.tile([69, NPAD], F32R, name="gt1", tag="gt1")
                nc.vector.tensor_copy(out=gt0[:, P:NPAD], in_=zpad[:, :])
                nc.vector.tensor_copy(out=gt1[:, P:NPAD], in_=zpad[:69, :])
                for sc, (s0, sn) in enumerate(SCH):
                    gt = (gt0, gt1)[sc]
                    for c, (p0, pn) in enumerate(PCH):
                        pt = psT.tile([sn, pn], F32, name="pt", tag="pt")
                        for ci in range(4):
                            nc.tensor.matmul(pt[:], lhsT=gcs[c][ci][:, s0:s0 + sn],
                                             rhs=ident[:pn, :pn],
                                             is_transpose=True,
                                             start=(ci == 0), stop=(ci == 3))
                        nc.scalar.copy(out=gt[:, p0:p0 + pn], in_=pt[:])
                gts.append((gt0, gt1))

                # ---- gather matmul: sxT[:, b*P:(b+1)*P] = (G @ x_b).T ----
                for mj in range(6):
                    pa = psA.tile([128, NPAD], F32, name="pa", tag="pa")
                    nc.tensor.matmul(pa[:], lhsT=xt[b][0][:, mj * 128:(mj + 1) * 128],
                                     rhs=gt0[:], start=True, stop=False)
                    nc.tensor.matmul(pa[:], lhsT=xt[b][1][:, mj * 128:(mj + 1) * 128],
                                     rhs=gt1[:], start=False, stop=True)
                    nc.scalar.copy(out=sxT[mj][:, b * P:(b + 1) * P],
                                   in_=pa[:, 0:P])

            # ---- projection matmul + output ----
            n_m = (Q + 127) // 128
            for mi in range(n_m):
                q0 = mi * 128
                mp = min(128, Q - q0)
                st = opool.tile([128, D], F32, name="ost", tag="ost")
                for n in range(2):
                    pb = psB.tile([128, 384], F32, name="pb", tag="pb")
                    for kc in range(6):
                        nc.tensor.matmul(pb[:mp, :], lhsT=sxT[kc][:, q0:q0 + mp],
                                         rhs=wv_sb[kc][:, n * 384:(n + 1) * 384],
                                         start=(kc == 0), stop=(kc == 5))
                    eng = nc.vector.tensor_copy if n == 0 else nc.scalar.copy
                    if n == 0:
                        nc.vector.tensor_copy(out=st[:mp, 0:384], in_=pb[:mp, :])
                    else:
                        nc.scalar.copy(out=st[:mp, 384:768], in_=pb[:mp, :])
                b0, poff = q0 // P, q0 % P
                seg1 = min(mp, P - poff)
                nc.sync.dma_start(
                    out=o_d[b0 * S + 1 + poff: b0 * S + 1 + poff + seg1, :],
                    in_=st[0:seg1, :])
                if seg1 < mp:
                    nc.sync.dma_start(
                        out=o_d[(b0 + 1) * S + 1: (b0 + 1) * S + 1 + mp - seg1, :],
                        in_=st[seg1:mp, :])

            # class-token rows = 1.0
            for b in range(BPC):
                nc.sync.dma_start(out=o_d[b * S:b * S + 1, :], in_=ones[b:b + 1, :])


def _build():
    global _NC
    if _NC is None:
        nc = bacc.Bacc("TRN2", target_bir_lowering=False, debug=False,
                       num_devices=N_CORES)
        _emit(nc)
        nc.compile()
        _NC = nc
    return _NC


def _pack_inputs(x, img_ids, Wv, avgs, std_devs, noise):
    x = np.ascontiguousarray(np.asarray(x, np.float32))
    wv = np.ascontiguousarray(np.asarray(Wv, np.float32))
    ids = np.asarray(img_ids).astype(np.int64)
    avgs = np.asarray(avgs, np.float32)
    std_devs = np.asarray(std_devs, np.float32)
    noise = np.asarray(noise, np.float32)
    in_maps = []
    for c in range(N_CORES):
        sl = slice(c * BPC, (c + 1) * BPC)
        xs = np.ascontiguousarray(x[sl].reshape(ROWS, D))
        a = avgs[ids[sl]]        # (BPC, 2, P)
        s = std_devs[ids[sl]]
        nz = noise[sl]
        pr = np.empty((P, 6, BPC), np.float32)
        pr[:, 0] = nz[:, 0].T
        pr[:, 1] = nz[:, 1].T
        pr[:, 2] = a[:, 0].T
        pr[:, 3] = a[:, 1].T
        pr[:, 4] = s[:, 0].T
        pr[:, 5] = s[:, 1].T
        in_maps.append({"x0": xs, "wv0": wv,
                        "pr0": np.ascontiguousarray(pr.reshape(P, 6 * BPC))})
    return in_maps


_RUNNER = None


def _get_runner():
    """Build the sharded PJRT callable once and cache it (the stock
    run_bass_kernel_spmd path re-jits every call)."""
    global _RUNNER
    if _RUNNER is not None:
        return _RUNNER
    import jax
    import numpy as _np
    from jax.experimental.shard_map import shard_map
    from jax.sharding import Mesh, PartitionSpec
    from concourse import bass2jax, mybir as _mybir

    nc = _build()
    bass2jax.install_neuronx_cc_hook()
    in_names, out_names, out_avals, zero_outs = [], [], [], []
    part_name = (nc.partition_id_tensor.name
                 if nc.partition_id_tensor else None)
    for alloc in nc.m.functions[0].allocations:
        if not isinstance(alloc, _mybir.MemoryLocationSet):
            continue
        name = alloc.memorylocations[0].name
        if alloc.kind == "ExternalInput":
            if name != part_name:
                in_names.append(name)
        elif alloc.kind == "ExternalOutput":
            shape = tuple(alloc.tensor_shape)
            dtype = _mybir.dt.np(alloc.dtype)
            out_names.append(name)
            out_avals.append(jax.core.ShapedArray(shape, dtype))
            zero_outs.append(_np.zeros(shape, dtype))
    n_params = len(in_names)
    all_names = in_names + out_names
    if part_name is not None:
        all_names = all_names + [part_name]
    donate = tuple(range(n_params, n_params + len(out_names)))

    def _body(*args):
        operands = list(args)
        if part_name is not None:
            operands.append(bass2jax.partition_id_tensor())
        outs = bass2jax._bass_exec_p.bind(
            *operands,
            out_avals=tuple(out_avals),
            in_names=tuple(all_names),
            out_names=tuple(out_names),
            lowering_input_output_aliases=(),
            sim_require_finite=True,
            sim_require_nnan=True,
            nc=nc,
        )
        return tuple(outs)

    devices = jax.devices()[:N_CORES]
    mesh = Mesh(_np.asarray(devices), ("core",))
    specs = (PartitionSpec("core"),) * (n_params + len(out_names))
    fn = jax.jit(
        shard_map(_body, mesh=mesh, in_specs=specs,
                  out_specs=(PartitionSpec("core"),) * len(out_names),
                  check_rep=False),
        donate_argnums=donate, keep_unused=True)

    def run(in_maps):
        concat_in = [
            _np.concatenate([_np.asarray(m[nm]) for m in in_maps], axis=0)
            for nm in in_names
        ]
        concat_zero = [
            _np.zeros((N_CORES * z.shape[0], *z.shape[1:]), z.dtype)
            for z in zero_outs
        ]
        arrs = fn(*concat_in, *concat_zero)
        return [
            {nm: _np.asarray(arrs[i]).reshape(N_CORES, *out_avals[i].shape)[c]
             for i, nm in enumerate(out_names)}
            for c in range(N_CORES)
        ]

    _RUNNER = run
    return run


class _Res:
    def __init__(self, results):
        self.results = results
        self.exec_time_ns = None


def run_cores(in_maps, trace=False):
    return _Res(_get_runner()(in_maps))


def kernel(x, img_ids, mask=None, Wq=None, bq=None, Wk=None, bk=None,
           Wv=None, bv=None, avgs=None, std_devs=None, noise=None,
           _trace=False, _results=None):
    in_maps = _pack_inputs(x, img_ids, Wv, avgs, std_devs, noise)
    res = run_cores(in_maps, trace=_trace)
    if _results is not None:
        _results.append(res)
    out = np.concatenate(
        [res.results[c]["o0"].reshape(BPC, S, D) for c in range(N_CORES)], axis=0)
    bv_np = np.asarray(bv, np.float32) if bv is not None else None
    if bv_np is not None and np.any(bv_np):
        # sample() is affine: add (sum_i w_i) * bv for the sampled rows.
        ids = np.asarray(img_ids).astype(np.int64)
        a = np.asarray(avgs, np.float32)[ids]
        sd = np.asarray(std_devs, np.float32)[ids]
        nz = np.asarray(noise, np.float32)
        kx = (nz[:, 0] - a[:, 0]) / sd[:, 0]
        ky = (nz[:, 1] - a[:, 1]) / sd[:, 1]
        fx1, fx2 = np.ceil(kx), np.floor(kx)
        fy1, fy2 = np.ceil(ky), np.floor(ky)
        wsum = ((1 - np.abs(fx1 - kx)) * (1 - np.abs(fy1 - ky))
                + (1 - np.abs(fx2 - kx)) * (1 - np.abs(fy1 - ky))
                + (1 - np.abs(fx1 - kx)) * (1 - np.abs(fy2 - ky))
                + (1 - np.abs(fx2 - kx)) * (1 - np.abs(fy2 - ky)))
        out[:, 1:, :] += wsum[:, :, None] * bv_np[None, None, :]
    return out
